# revision 6
# baseline (speedup 1.0000x reference)
"""Trainium2 Bass kernel: row-wise Dempster-Shafer combination of two
Dirichlet opinions (C = 21 classes, N = 2097152 rows).

The reference computes, per row:
    S_k = sum(alpha_k);  b_k = (alpha_k - 1)/S_k;  u_k = C/S_k
    K = sum(b0)*sum(b1) - dot(b0, b1);  denom = 1 - K
    b = (b0*b1 + b0*u1 + b1*u0)/denom;  u = u0*u1/denom
    alpha_out = b*(C/u) + 1

Algebraically `denom` cancels out of alpha_out entirely and the whole map
collapses to the elementwise closed form

    alpha_out = (alpha1 + C-1) * (alpha2 + C-1) / C - (C-1)

(max rel err vs the fp32 reference ~3e-6 — pure rounding).  So the kernel
is a pure streaming elementwise pipeline: rows are sharded across the 8
NeuronCores (data parallel, no communication), each core streams its
contiguous 22 MB block of both inputs through SBUF in 8 chunks, applies
three fused elementwise ops (1 ACT + 2 DVE), and writes the result back.
Memory-bound by design: ~66 MB of HBM traffic per core.
"""

import numpy as np

import concourse.bacc as bacc
import concourse.bass as bass
import concourse.tile as tile
from concourse import mybir
from concourse.bass_utils import run_bass_kernel_spmd

N_CORES = 8
N_ROWS = 2097152
C = 21
PER = N_ROWS // N_CORES          # 262144 rows per core
ELEMS = PER * C                  # 5505024 f32 elements per tensor per core
P = 128                          # SBUF partitions
FREE = ELEMS // P                # 43008 contiguous f32 per partition
F = 5376                         # chunk width (21 * 256): 8 chunks/core, 2.75 MB DMAs
NCHUNK = FREE // F

_nc = None


def _build():
    global _nc
    if _nc is not None:
        return _nc
    # Bacc (not raw Bass): its compile() runs generate_event_semaphores,
    # which legalizes multi-sem dependencies to the HW limit of one sync
    # wait per instruction by inserting EventSemaphore instructions.
    nc = bacc.Bacc(None)
    a1 = nc.dram_tensor("alpha1", [P, FREE], mybir.dt.float32, kind="ExternalInput")
    a2 = nc.dram_tensor("alpha2", [P, FREE], mybir.dt.float32, kind="ExternalInput")
    out = nc.dram_tensor("out", [P, FREE], mybir.dt.float32, kind="ExternalOutput")

    with tile.TileContext(nc) as tc:
        with (
            tc.tile_pool(name="t1", bufs=3) as pool1,
            tc.tile_pool(name="t2", bufs=3) as pool2,
        ):
            for i in range(NCHUNK):
                sl = slice(i * F, (i + 1) * F)
                t1 = pool1.tile([P, F], mybir.dt.float32)
                t2 = pool2.tile([P, F], mybir.dt.float32)
                nc.sync.dma_start(out=t1[:], in_=a1[:, sl])
                nc.sync.dma_start(out=t2[:], in_=a2[:, sl])
                # All compute on DVE: the NEFF encoding allows only ONE
                # sync-wait per instruction, and a single engine makes every
                # in-engine dependency ride the same semaphore (mergeable),
                # so each op waits on at most one sem.
                # t1 = a1 + 20                 (tensor_scalar, 2x mode)
                nc.vector.tensor_scalar_add(t1[:], t1[:], float(C - 1))
                # t2 = (a2 + 20) * (1/21)      (fused 2-scalar-op, 2x mode)
                nc.vector.tensor_scalar(
                    t2[:], t2[:], float(C - 1), float(1.0 / C),
                    op0=mybir.AluOpType.add, op1=mybir.AluOpType.mult,
                )
                # t1 = t1 * t2                 (tensor_tensor, 1x mode)
                nc.vector.tensor_mul(t1[:], t1[:], t2[:])
                # t1 = t1 - 20                 (tensor_scalar, 2x mode)
                nc.vector.tensor_scalar_add(t1[:], t1[:], float(-(C - 1)))
                nc.sync.dma_start(out=out[:, sl], in_=t1[:])
    # Bacc defers register allocation etc. to compile(), which finalize()
    # runs; the bass2jax exec path serializes without finalizing.
    nc.finalize()
    _nc = nc
    return nc


def _run(alpha1, alpha2, trace=False, **kwargs):
    nc = _build()
    alpha1 = np.ascontiguousarray(np.asarray(alpha1, dtype=np.float32))
    alpha2 = np.ascontiguousarray(np.asarray(alpha2, dtype=np.float32))
    in_maps = []
    for c in range(N_CORES):
        blk = slice(c * PER, (c + 1) * PER)
        in_maps.append({
            "alpha1": alpha1[blk].reshape(P, FREE),
            "alpha2": alpha2[blk].reshape(P, FREE),
        })
    res = run_bass_kernel_spmd(nc, in_maps, list(range(N_CORES)), trace=trace, **kwargs)
    full = np.empty((N_ROWS, C), dtype=np.float32)
    for c in range(N_CORES):
        full[c * PER:(c + 1) * PER] = res.results[c]["out"].reshape(PER, C)
    return full, res


def kernel(alpha1, alpha2):
    return _run(alpha1, alpha2)[0]


# revision 8
# speedup vs baseline: 33.9264x; 33.9264x over previous
"""Trainium2 Bass kernel: row-wise Dempster-Shafer combination of two
Dirichlet opinions (C = 21 classes, N = 2097152 rows).

The reference computes, per row:
    S_k = sum(alpha_k);  b_k = (alpha_k - 1)/S_k;  u_k = C/S_k
    K = sum(b0)*sum(b1) - dot(b0, b1);  denom = 1 - K
    b = (b0*b1 + b0*u1 + b1*u0)/denom;  u = u0*u1/denom
    alpha_out = b*(C/u) + 1

Algebraically `denom` cancels out of alpha_out entirely and the whole map
collapses to the elementwise closed form

    alpha_out = (alpha1 + C-1) * (alpha2 + C-1) / C - (C-1)

(max rel err vs the fp32 reference ~3e-6 — pure rounding).  So the kernel
is a pure streaming elementwise pipeline: rows are sharded across the 8
NeuronCores (data parallel, no communication), each core streams its
contiguous 22 MB block of both inputs through SBUF in 8 chunks, applies
three fused elementwise ops (1 ACT + 2 DVE), and writes the result back.
Memory-bound by design: ~66 MB of HBM traffic per core.
"""

import numpy as np

import concourse.bacc as bacc
import concourse.bass as bass
import concourse.tile as tile
from concourse import mybir
from concourse.bass_utils import run_bass_kernel_spmd

N_CORES = 8
N_ROWS = 2097152
C = 21
PER = N_ROWS // N_CORES          # 262144 rows per core
ELEMS = PER * C                  # 5505024 f32 elements per tensor per core
P = 128                          # SBUF partitions
FREE = ELEMS // P                # 43008 contiguous f32 per partition
F = 5376                         # chunk width (21 * 256): 8 chunks/core, 2.75 MB DMAs
NCHUNK = FREE // F

_nc_cache = {}


def _build(repeats=1):
    """Build the Bass program. `repeats` re-runs the whole streaming pipeline
    N times inside one NEFF — used by the test harness to measure pure device
    time as a slope between two repeat counts (cancels dispatch overhead)."""
    if repeats in _nc_cache:
        return _nc_cache[repeats]
    # Bacc (not raw Bass): its compile() runs generate_event_semaphores,
    # which legalizes multi-sem dependencies to the HW limit of one sync
    # wait per instruction by inserting EventSemaphore instructions.
    nc = bacc.Bacc(None)
    a1 = nc.dram_tensor("alpha1", [P, FREE], mybir.dt.float32, kind="ExternalInput")
    a2 = nc.dram_tensor("alpha2", [P, FREE], mybir.dt.float32, kind="ExternalInput")
    out = nc.dram_tensor("out", [P, FREE], mybir.dt.float32, kind="ExternalOutput")

    with tile.TileContext(nc) as tc:
        with (
            tc.tile_pool(name="t1", bufs=3) as pool1,
            tc.tile_pool(name="t2", bufs=3) as pool2,
        ):
            for i in range(NCHUNK * repeats):
                i = i % NCHUNK
                sl = slice(i * F, (i + 1) * F)
                t1 = pool1.tile([P, F], mybir.dt.float32)
                t2 = pool2.tile([P, F], mybir.dt.float32)
                nc.sync.dma_start(out=t1[:], in_=a1[:, sl])
                nc.sync.dma_start(out=t2[:], in_=a2[:, sl])
                # All compute on DVE: the NEFF encoding allows only ONE
                # sync-wait per instruction, and a single engine makes every
                # in-engine dependency ride the same semaphore (mergeable),
                # so each op waits on at most one sem.
                # t1 = a1 + 20                 (tensor_scalar, 2x mode)
                nc.vector.tensor_scalar_add(t1[:], t1[:], float(C - 1))
                # t2 = (a2 + 20) * (1/21)      (fused 2-scalar-op, 2x mode)
                nc.vector.tensor_scalar(
                    t2[:], t2[:], float(C - 1), float(1.0 / C),
                    op0=mybir.AluOpType.add, op1=mybir.AluOpType.mult,
                )
                # t1 = t1 * t2                 (tensor_tensor, 1x mode)
                nc.vector.tensor_mul(t1[:], t1[:], t2[:])
                # t1 = t1 - 20                 (tensor_scalar, 2x mode)
                nc.vector.tensor_scalar_add(t1[:], t1[:], float(-(C - 1)))
                nc.sync.dma_start(out=out[:, sl], in_=t1[:])
    # Bacc defers register allocation etc. to compile(), which finalize()
    # runs; the bass2jax exec path serializes without finalizing.
    nc.finalize()
    _nc_cache[repeats] = nc
    return nc


def _run(alpha1, alpha2, trace=False, repeats=1, **kwargs):
    nc = _build(repeats)
    alpha1 = np.ascontiguousarray(np.asarray(alpha1, dtype=np.float32))
    alpha2 = np.ascontiguousarray(np.asarray(alpha2, dtype=np.float32))
    in_maps = []
    for c in range(N_CORES):
        blk = slice(c * PER, (c + 1) * PER)
        in_maps.append({
            "alpha1": alpha1[blk].reshape(P, FREE),
            "alpha2": alpha2[blk].reshape(P, FREE),
        })
    res = run_bass_kernel_spmd(nc, in_maps, list(range(N_CORES)), trace=trace, **kwargs)
    full = np.empty((N_ROWS, C), dtype=np.float32)
    for c in range(N_CORES):
        full[c * PER:(c + 1) * PER] = res.results[c]["out"].reshape(PER, C)
    return full, res


def kernel(alpha1, alpha2):
    return _run(alpha1, alpha2)[0]


# revision 10
# speedup vs baseline: 40.8490x; 1.2040x over previous
"""Trainium2 Bass kernel: row-wise Dempster-Shafer combination of two
Dirichlet opinions (C = 21 classes, N = 2097152 rows).

The reference computes, per row:
    S_k = sum(alpha_k);  b_k = (alpha_k - 1)/S_k;  u_k = C/S_k
    K = sum(b0)*sum(b1) - dot(b0, b1);  denom = 1 - K
    b = (b0*b1 + b0*u1 + b1*u0)/denom;  u = u0*u1/denom
    alpha_out = b*(C/u) + 1

Algebraically `denom` cancels out of alpha_out entirely and the whole map
collapses to the elementwise closed form

    alpha_out = (alpha1 + C-1) * (alpha2 + C-1) / C - (C-1)

(max rel err vs the fp32 reference ~3e-6 — pure rounding).  So the kernel
is a pure streaming elementwise pipeline: rows are sharded across the 8
NeuronCores (data parallel, no communication), each core streams its
contiguous 22 MB block of both inputs through SBUF in 8 chunks, applies
three fused elementwise ops (1 ACT + 2 DVE), and writes the result back.
Memory-bound by design: ~66 MB of HBM traffic per core.
"""

import numpy as np

import concourse.bacc as bacc
import concourse.bass as bass
import concourse.tile as tile
from concourse import mybir
from concourse.bass_utils import run_bass_kernel_spmd

N_CORES = 8
N_ROWS = 2097152
C = 21
PER = N_ROWS // N_CORES          # 262144 rows per core
ELEMS = PER * C                  # 5505024 f32 elements per tensor per core
P = 128                          # SBUF partitions
FREE = ELEMS // P                # 43008 contiguous f32 per partition
F = 5376                         # chunk width (21 * 256): 8 chunks/core, 2.75 MB DMAs
NCHUNK = FREE // F

_nc_cache = {}


def _build(repeats=1):
    """Build the Bass program. `repeats` re-runs the whole streaming pipeline
    N times inside one NEFF — used by the test harness to measure pure device
    time as a slope between two repeat counts (cancels dispatch overhead)."""
    if repeats in _nc_cache:
        return _nc_cache[repeats]
    # Bacc (not raw Bass): its compile() runs generate_event_semaphores,
    # which legalizes multi-sem dependencies to the HW limit of one sync
    # wait per instruction by inserting EventSemaphore instructions.
    nc = bacc.Bacc(None)
    a1 = nc.dram_tensor("alpha1", [P, FREE], mybir.dt.float32, kind="ExternalInput")
    a2 = nc.dram_tensor("alpha2", [P, FREE], mybir.dt.float32, kind="ExternalInput")
    out = nc.dram_tensor("out", [P, FREE], mybir.dt.float32, kind="ExternalOutput")

    with tile.TileContext(nc) as tc:
        with (
            tc.tile_pool(name="t1", bufs=4) as pool1,
            tc.tile_pool(name="t2", bufs=4) as pool2,
        ):
            for i in range(NCHUNK * repeats):
                i = i % NCHUNK
                sl = slice(i * F, (i + 1) * F)
                t1 = pool1.tile([P, F], mybir.dt.float32)
                t2 = pool2.tile([P, F], mybir.dt.float32)
                nc.sync.dma_start(out=t1[:], in_=a1[:, sl])
                nc.sync.dma_start(out=t2[:], in_=a2[:, sl])
                # All compute on DVE: the NEFF encoding allows only ONE
                # sync-wait per instruction, and a single engine makes every
                # in-engine dependency ride the same semaphore (mergeable),
                # so each op waits on at most one sem.
                # t1 = a1 + 20                 (tensor_scalar, 2x mode)
                nc.vector.tensor_scalar_add(t1[:], t1[:], float(C - 1))
                # t2 = (a2 + 20) * (1/21)      (fused 2-scalar-op, 2x mode)
                nc.vector.tensor_scalar(
                    t2[:], t2[:], float(C - 1), float(1.0 / C),
                    op0=mybir.AluOpType.add, op1=mybir.AluOpType.mult,
                )
                # t1 = t1 * t2                 (tensor_tensor, 1x mode)
                nc.vector.tensor_mul(t1[:], t1[:], t2[:])
                # t1 = t1 - 20                 (tensor_scalar, 2x mode)
                nc.vector.tensor_scalar_add(t1[:], t1[:], float(-(C - 1)))
                # Store via the ACT sequencer's HWDGE ring (qActDynamicHW):
                # stores wait on compute, and on the SP ring that wait would
                # block the in-order sequencer from issuing later loads.
                nc.scalar.dma_start(out=out[:, sl], in_=t1[:])
    # Bacc defers register allocation etc. to compile(), which finalize()
    # runs; the bass2jax exec path serializes without finalizing.
    nc.finalize()
    _nc_cache[repeats] = nc
    return nc


def _run(alpha1, alpha2, trace=False, repeats=1, **kwargs):
    nc = _build(repeats)
    alpha1 = np.ascontiguousarray(np.asarray(alpha1, dtype=np.float32))
    alpha2 = np.ascontiguousarray(np.asarray(alpha2, dtype=np.float32))
    in_maps = []
    for c in range(N_CORES):
        blk = slice(c * PER, (c + 1) * PER)
        in_maps.append({
            "alpha1": alpha1[blk].reshape(P, FREE),
            "alpha2": alpha2[blk].reshape(P, FREE),
        })
    res = run_bass_kernel_spmd(nc, in_maps, list(range(N_CORES)), trace=trace, **kwargs)
    full = np.empty((N_ROWS, C), dtype=np.float32)
    for c in range(N_CORES):
        full[c * PER:(c + 1) * PER] = res.results[c]["out"].reshape(PER, C)
    return full, res


def kernel(alpha1, alpha2):
    return _run(alpha1, alpha2)[0]
